# revision 5
# baseline (speedup 1.0000x reference)
"""Trainium2 Bass kernel for nn_MoE_CNN_94489281288 (moe_routing).

Model (per batch element):
  features = concat([fs, fp], -1).T                       # [CH=128, L=2048]
  gate: Conv1d(128,64,k=32,s=8) -> relu -> max_t -> Linear(64,4) -> softmax
  experts (x4, stacked): Conv1d(128,64,32,8) -> relu -> max_t
          -> Lin(64,32) -> relu -> Lin(32,16) -> relu
  out = log_softmax(fc(sum_e gate_e * h_e))               # [B,10], plus gate [B,4]

Strategy: pure data parallel over 8 NeuronCores (8 batch elements each).
On-core dataflow:
  - DMA raw [l, c] chunks, PE-transpose to [c, l], DVE-scatter into a
    phase-major layout feat[c, s, t] (l = 8t + s) so that each conv tap k
    is a CONTIGUOUS slice feat[:, k%8, k//8 : k//8+N]  (fp32r requirement).
  - Conv for gate+all experts as one 320-channel output, batch-PAIRED in
    the moving dim (N=510 window columns covers 2 batch elements with 3
    garbage columns between and 1 at the end).
  - fp32r matmuls: 128-contraction per tap, 32 taps accumulate in PSUM.
  - relu+max fused as reduce_max over valid columns then Relu(max + bias).
  - Tiny gate/expert-MLP/fc tail in fp32 on [*, 8] tiles (batch on free
    dim); partition-dim softmax/log_softmax via ones-matmul reductions.
"""
import sys

sys.path.insert(0, "/opt/trn_rl_repo")

import numpy as np

import concourse.bacc as bacc
import concourse.tile as tile
import concourse.mybir as mybir

F32 = mybir.dt.float32
F32R = mybir.dt.float32r
AF = mybir.ActivationFunctionType

# problem shapes (hardcoded per contract)
B, L, CH, E, HID, OUT = 64, 2048, 128, 4, 64, 10
K, STRIDE = 32, 8
NCORES = 8
BL = B // NCORES          # 8 local batch elements per core
NP = BL // 2              # 4 batch pairs
LP = L // 8               # 256 t-slots per batch element per phase
TT = 2 * LP               # 512 t-slots per pair
TP = TT + 4               # padded t-dim (taps read up to t = k//8 + 509)
NW = 510                  # moving columns per conv matmul (even, fp32r)
LV = (L - K) // STRIDE + 1  # 253 valid windows per batch element
NCH = HID + E * HID       # 320 conv output channels (gate + experts)
GROUPS = ((0, 128), (128, 128), (256, 64))  # out-channel PSUM groups


def build_nc():
    nc = bacc.Bacc(trn_type="TRN2", target_bir_lowering=False, debug=False)

    # inputs (per-core)
    fs_d = nc.dram_tensor("fs", [BL, L, CH // 2], F32, kind="ExternalInput").ap()
    fp_d = nc.dram_tensor("fp", [BL, L, CH // 2], F32, kind="ExternalInput").ap()
    # conv weights [c, k, o] for all 320 out-channels, consumed as fp32r
    cw_d = nc.dram_tensor("cw", [CH, K, NCH], F32R, kind="ExternalInput").ap()
    ident_d = nc.dram_tensor("ident", [128, 128], F32, kind="ExternalInput").ap()
    cb_d = nc.dram_tensor("cb", [128, 3], F32, kind="ExternalInput").ap()
    glw_d = nc.dram_tensor("glw", [HID, E], F32, kind="ExternalInput").ap()
    glb_d = nc.dram_tensor("glb", [E, 1], F32, kind="ExternalInput").ap()
    w1_d = nc.dram_tensor("w1", [2 * HID, E, 32], F32, kind="ExternalInput").ap()
    b1_d = nc.dram_tensor("b1", [32, E], F32, kind="ExternalInput").ap()
    w2_d = nc.dram_tensor("w2", [32, E, 16], F32, kind="ExternalInput").ap()
    b2_d = nc.dram_tensor("b2", [16, E], F32, kind="ExternalInput").ap()
    fcw_d = nc.dram_tensor("fcw", [16, OUT], F32, kind="ExternalInput").ap()
    fcb_d = nc.dram_tensor("fcb", [OUT, 1], F32, kind="ExternalInput").ap()
    bsel_d = nc.dram_tensor("bsel", [E, 128], F32, kind="ExternalInput").ap()
    ssel_d = nc.dram_tensor("ssel", [128, 16], F32, kind="ExternalInput").ap()
    ones_d = nc.dram_tensor("ones", [16, 16], F32, kind="ExternalInput").ap()

    out_d = nc.dram_tensor("out", [BL, OUT], F32, kind="ExternalOutput").ap()
    gout_d = nc.dram_tensor("gout", [BL, E], F32, kind="ExternalOutput").ap()

    with tile.TileContext(nc) as tc:
        with (
            tc.tile_pool(name="persist", bufs=1) as persist,
            tc.tile_pool(name="feats", bufs=2) as feats,
            tc.tile_pool(name="raws", bufs=3) as raws,
            tc.tile_pool(name="work", bufs=2) as work,
        ):
            # ---- one-time loads -------------------------------------------------
            w_sb = persist.tile([CH, K, NCH], F32R)
            nc.sync.dma_start(out=w_sb, in_=cw_d)
            ident = persist.tile([128, 128], F32)
            nc.sync.dma_start(out=ident, in_=ident_d)
            cb = persist.tile([128, 3], F32)
            nc.sync.dma_start(out=cb, in_=cb_d)
            glw = persist.tile([HID, E], F32)
            nc.sync.dma_start(out=glw, in_=glw_d)
            glb = persist.tile([E, 1], F32)
            nc.sync.dma_start(out=glb, in_=glb_d)
            w1 = persist.tile([2 * HID, E, 32], F32)
            nc.sync.dma_start(out=w1, in_=w1_d)
            b1 = persist.tile([32, E], F32)
            nc.sync.dma_start(out=b1, in_=b1_d)
            w2 = persist.tile([32, E, 16], F32)
            nc.sync.dma_start(out=w2, in_=w2_d)
            b2 = persist.tile([16, E], F32)
            nc.sync.dma_start(out=b2, in_=b2_d)
            fcw = persist.tile([16, OUT], F32)
            nc.sync.dma_start(out=fcw, in_=fcw_d)
            fcb = persist.tile([OUT, 1], F32)
            nc.sync.dma_start(out=fcb, in_=fcb_d)
            bsel = persist.tile([E, 128], F32)
            nc.sync.dma_start(out=bsel, in_=bsel_d)
            ssel = persist.tile([128, 16], F32)
            nc.sync.dma_start(out=ssel, in_=ssel_d)
            ones = persist.tile([16, 16], F32)
            nc.sync.dma_start(out=ones, in_=ones_d)

            # conv max results per group, [128, BL] (g2 uses rows 0:64)
            cmax = [persist.tile([128, BL], F32, tag=f"cmax{g}", name=f"cmax{g}") for g in range(3)]

            # ---- conv over 4 batch pairs ---------------------------------------
            with (
                tc.tile_pool(name="ptp", bufs=2, space="PSUM") as ptp,
                tc.tile_pool(name="pconv", bufs=2, space="PSUM") as pconv,
            ):
                for pair in range(NP):
                    featp = feats.tile([CH, 8, TP], F32R, tag="featp")
                    # pad columns (read by tap k//8=3 at garbage col 509 only)
                    nc.vector.tensor_copy(
                        featp[:, :, TT:TP],
                        ident[:, 0 : 8 * (TP - TT)].rearrange(
                            "p (s t) -> p s t", s=8
                        ),
                    )
                    for j in range(2):  # batch element within pair
                        b = 2 * pair + j
                        raw = raws.tile([128, L // 128, CH], F32, tag="raw")
                        nc.sync.dma_start(
                            out=raw[:, :, 0 : CH // 2],
                            in_=fs_d[b].rearrange("(n p) c -> p n c", p=128),
                        )
                        nc.sync.dma_start(
                            out=raw[:, :, CH // 2 : CH],
                            in_=fp_d[b].rearrange("(n p) c -> p n c", p=128),
                        )
                        for n in range(L // 128):
                            tps = ptp.tile([128, 128], F32, tag="tps")
                            nc.tensor.transpose(tps, raw[:, n, :], ident)
                            # l = 128n + jj = 8t + s; t = j*256 + 16n + jj//8
                            nc.vector.tensor_copy(
                                featp[:, :, j * LP + 16 * n : j * LP + 16 * n + 16],
                                tps.rearrange("p (t s) -> p s t", s=8),
                            )

                    psums = []
                    for g, (g0, gm) in enumerate(GROUPS):
                        pg = pconv.tile([gm, 512], F32, tag=f"pg{g}")
                        for k in range(K):
                            nc.tensor.matmul(
                                pg[:, 0:NW],
                                w_sb[:, k, g0 : g0 + gm],
                                featp[:, k % 8, k // 8 : k // 8 + NW],
                                start=(k == 0),
                                stop=(k == K - 1),
                            )
                        psums.append(pg)
                    for g, (g0, gm) in enumerate(GROUPS):
                        for j in range(2):
                            nc.vector.reduce_max(
                                out=cmax[g][:gm, 2 * pair + j : 2 * pair + j + 1],
                                in_=psums[g][:, j * LP : j * LP + LV],
                                axis=mybir.AxisListType.X,
                            )

            # conv bias + relu (bias constant over t commutes with max)
            for g, (g0, gm) in enumerate(GROUPS):
                nc.scalar.activation(
                    cmax[g][:gm, :], cmax[g][:gm, :], AF.Relu, bias=cb[:gm, g : g + 1]
                )

            # ---- tail: gate softmax, expert MLPs, mix, fc, log_softmax ---------
            # hidden slices per expert: e0=cmax0[64:], e1=cmax1[:64],
            # e2=cmax1[64:], e3=cmax2[:64]; gate = cmax0[:64]
            h_e = [
                cmax[0][64:128, :],
                cmax[1][0:64, :],
                cmax[1][64:128, :],
                cmax[2][0:64, :],
            ]
            with tc.tile_pool(name="ptail", bufs=3, space="PSUM") as ptail:
                # gate logits -> softmax (partition-dim, via ones-matmuls)
                psg = ptail.tile([E, BL], F32, tag="tt")
                nc.tensor.matmul(psg, glw, cmax[0][0:64, :], start=True, stop=True)
                expg = work.tile([E, BL], F32, tag="expg")
                nc.scalar.activation(expg, psg, AF.Exp, bias=glb)
                pssum = ptail.tile([1, BL], F32, tag="tt")
                nc.tensor.matmul(pssum, ones[0:E, 0:1], expg, start=True, stop=True)
                rec = work.tile([1, BL], F32, tag="rec")
                nc.vector.reciprocal(rec, pssum)
                psb = ptail.tile([E, BL], F32, tag="tt")
                nc.tensor.matmul(psb, ones[0:1, 0:E], rec, start=True, stop=True)
                gsm = work.tile([E, BL], F32, tag="gsm")
                nc.vector.tensor_mul(gsm, expg, psb)
                nc.sync.dma_start(out=gout_d.transpose([1, 0]), in_=gsm)

                # broadcast gate to expert rows (16 per expert, 32-strided
                # because engines only support start partitions 0/32/64/96)
                psgb = ptail.tile([128, BL], F32, tag="tgb")
                nc.tensor.matmul(psgb, bsel, gsm, start=True, stop=True)

                y2s = work.tile([128, BL], F32, tag="y2s")
                nc.vector.memset(y2s, 0.0)
                h_base = [64, 0, 64, 0]  # partition base of each expert's hidden
                for e in range(E):
                    psy1 = ptail.tile([32, BL], F32, tag="tt")
                    hb = h_base[e]
                    nc.tensor.matmul(psy1, w1[hb : hb + HID, e, :], h_e[e],
                                     start=True, stop=True)
                    y1 = work.tile([32, BL], F32, tag="y1")
                    nc.scalar.activation(y1, psy1, AF.Relu, bias=b1[:, e : e + 1])
                    psy2 = ptail.tile([16, BL], F32, tag="tt")
                    nc.tensor.matmul(psy2, w2[:, e, :], y1, start=True, stop=True)
                    nc.scalar.activation(
                        y2s[32 * e : 32 * e + 16, :],
                        psy2,
                        AF.Relu,
                        bias=b2[:, e : e + 1],
                    )

                prod = work.tile([128, BL], F32, tag="prod")
                nc.vector.tensor_mul(prod, y2s, psgb)
                psmx = ptail.tile([16, BL], F32, tag="tt")
                nc.tensor.matmul(psmx, ssel, prod, start=True, stop=True)
                mixed = work.tile([16, BL], F32, tag="mixed")
                nc.vector.tensor_copy(mixed, psmx)

                psf = ptail.tile([OUT, BL], F32, tag="tt")
                nc.tensor.matmul(psf, fcw, mixed, start=True, stop=True)
                flog = work.tile([OUT, BL], F32, tag="flog")
                nc.scalar.activation(flog, psf, AF.Identity, bias=fcb)
                expf = work.tile([OUT, BL], F32, tag="expf")
                nc.scalar.activation(expf, flog, AF.Exp)
                psz = ptail.tile([1, BL], F32, tag="tt")
                nc.tensor.matmul(psz, ones[0:OUT, 0:1], expf, start=True, stop=True)
                logz = work.tile([1, BL], F32, tag="logz")
                nc.scalar.activation(logz, psz, AF.Ln)
                psbz = ptail.tile([OUT, BL], F32, tag="tt")
                nc.tensor.matmul(
                    psbz, ones[0:1, 0:OUT], logz, start=True, stop=True
                )
                outt = work.tile([OUT, BL], F32, tag="outt")
                nc.vector.tensor_sub(outt, flog, psbz)
                nc.sync.dma_start(out=out_d.transpose([1, 0]), in_=outt)

    nc.compile()
    return nc


def prep_weights(gate_cw, gate_cb, gate_lw, gate_lb, exp_cw, exp_cb, exp_w1,
                 exp_b1, exp_w2, exp_b2, fc_w, fc_b):
    """Host-side layout prep of the (tiny, replicated) parameters."""
    f32 = np.float32
    w_all = np.concatenate(
        [np.asarray(gate_cw, f32), np.asarray(exp_cw, f32).reshape(E * HID, CH, K)], 0
    )  # [320, c, k]
    cw = np.ascontiguousarray(w_all.transpose(1, 2, 0))  # [c, k, o]
    b_all = np.concatenate(
        [np.asarray(gate_cb, f32), np.asarray(exp_cb, f32).reshape(E * HID)], 0
    )  # [320]
    cb = np.zeros((128, 3), f32)
    cb[:, 0] = b_all[0:128]
    cb[:, 1] = b_all[128:256]
    cb[0:64, 2] = b_all[256:320]
    bsel = np.zeros((E, 128), f32)
    for e in range(E):
        bsel[e, 32 * e : 32 * e + 16] = 1.0
    ssel = np.zeros((128, 16), f32)
    for e in range(E):
        ssel[32 * e : 32 * e + 16, :] = np.eye(16, dtype=f32)
    return {
        "cw": cw,
        "ident": np.eye(128, dtype=f32),
        "cb": cb,
        "glw": np.ascontiguousarray(np.asarray(gate_lw, f32).T),        # [64, 4]
        "glb": np.asarray(gate_lb, f32).reshape(E, 1),
        "w1": np.ascontiguousarray(np.tile(np.asarray(exp_w1, f32).transpose(2, 0, 1), (2, 1, 1))),  # [128, E, 32] duplicated rows
        "b1": np.ascontiguousarray(np.asarray(exp_b1, f32).T),          # [32, E]
        "w2": np.ascontiguousarray(np.asarray(exp_w2, f32).transpose(2, 0, 1)),  # [32, E, 16]
        "b2": np.ascontiguousarray(np.asarray(exp_b2, f32).T),          # [16, E]
        "fcw": np.ascontiguousarray(np.asarray(fc_w, f32).T),           # [16, 10]
        "fcb": np.asarray(fc_b, f32).reshape(OUT, 1),
        "bsel": bsel,
        "ssel": ssel,
        "ones": np.ones((16, 16), f32),
    }


_NC_CACHE = None


def _get_nc():
    global _NC_CACHE
    if _NC_CACHE is None:
        _NC_CACHE = build_nc()
    return _NC_CACHE


def kernel(fs, fp, gate_cw, gate_cb, gate_lw, gate_lb, exp_cw, exp_cb,
           exp_w1, exp_b1, exp_w2, exp_b2, fc_w, fc_b):
    from concourse.bass_utils import run_bass_kernel_spmd

    nc = _get_nc()
    wmap = prep_weights(gate_cw, gate_cb, gate_lw, gate_lb, exp_cw, exp_cb,
                        exp_w1, exp_b1, exp_w2, exp_b2, fc_w, fc_b)
    fs = np.asarray(fs, np.float32)
    fp = np.asarray(fp, np.float32)
    in_maps = []
    for c in range(NCORES):
        m = dict(wmap)
        m["fs"] = np.ascontiguousarray(fs[c * BL : (c + 1) * BL])
        m["fp"] = np.ascontiguousarray(fp[c * BL : (c + 1) * BL])
        in_maps.append(m)
    res = run_bass_kernel_spmd(nc, in_maps, core_ids=list(range(NCORES)))
    out = np.concatenate([res.results[c]["out"] for c in range(NCORES)], axis=0)
    gout = np.concatenate([res.results[c]["gout"] for c in range(NCORES)], axis=0)
    return out, gout
